# revision 18
# baseline (speedup 1.0000x reference)
"""Trainium2 Bass kernel for the ATCA/TCA spiking cluster loss.

Contract: kernel(**inputs) takes FULL inputs (vmem [256,500,128] f32,
vlastmem [256,500,128] f32 (unused by the math), labels_t [256,128] i32,
ratio scalar (unused)) and returns (loss: f32 scalar, spike_output
[256,128] f32), exactly like the reference.

Strategy (data-parallel over batch, 8 NeuronCores):
  - host: shard vmem/labels along batch (32 batches/core); transpose each
    batch slice to [128 neurons, 532 t] (time on the SBUF free axis; col 0
    and cols 501..531 are -1 pads = "no spike", covering t = -1 and
    t = 500..527 so all shifted views and 11-blocks stay in range)
  - device (per core, per batch tile [128, 532]):
      spk' = (v < 0) = Sign(Relu(-v))
      tsls = scan state = spk'*(state+1)        (time since last spike)
      is_start[t] = (tsls[t-1] - 10 >= tsls[t]); nclus = sum(is_start)
      mask[t] = (tsls[t+10] <= 20)              (spike within [t-10, t+10])
      m0 = max(v - 1e30*mask); full0 = m0 < -1e29
      excess = full0 ? vmax : -m0               (valid since every pair spikes:
                                                 the argmax win lies inside mask)
      cluster maxima at BLOCK level: gaps between clusters are >= 11 steps,
      so an 11-block intersects at most one cluster. bmax = per-block max of
      v (48 blocks over t=0..527); bstart = per-block OR of is_start, with a
      forced fake start at t=506 (block 46 boundary) closing the last real
      cluster; a 47-wide segmented-max scan over bmax + top8 of the
      end-masked values yields the per-cluster maxima (negated, ascending),
      with at most one positive "fake" entry from the pre-first-spike region
      offset-corrected during selection (off = 1 - bstart[0]).
      contrib = label > nclus ? excess : (label < nclus ? deficit : 0)
  - device reduces contribs to [128] partials; host sums 8x128 partials and
    concatenates spike counts.

Engine split: DVE gets the scans, compares and reductions; Pool (gpsimd)
gets the plain tensor_tensor ops; ACT gets the affine/Relu/Sign ops; spare
DMA queues move the tiny top8 tiles into the stats buffer. walrus on this
toolchain embeds at most one sync-wait per TPB/DMA instruction (none on
InstMax); _fix_wait_overflow moves overflow waits onto injected NoOps.
"""
import sys

sys.path.insert(0, "/opt/trn_rl_repo")

import numpy as np
import concourse.bass as bass
import concourse.tile as tile
from concourse import mybir
from concourse.bass_utils import run_bass_kernel_spmd

AF = mybir.AluOpType
F32 = mybir.dt.float32

B, T, N = 256, 500, 128
NCORES = 8
NB = B // NCORES  # 32 batch elements per core
SENT = 64.0
BIG = 1e30
W = 532           # padded time width: col0 = t=-1, cols 1..529+ = t 0..527+


def _build(nb: int) -> bass.Bass:
    ACT = mybir.ActivationFunctionType
    nc = bass.Bass()
    vt = nc.dram_tensor("vt", [nb, 128, W], F32, kind="ExternalInput")
    labt = nc.dram_tensor("labt", [128, nb], F32, kind="ExternalInput")
    out = nc.dram_tensor("out", [128, nb + 1], F32, kind="ExternalOutput")

    with tile.TileContext(nc) as tc:
        with (
            tc.tile_pool(name="work", bufs=8) as work,
            tc.tile_pool(name="sing", bufs=1) as sing,
        ):
            lab_s = sing.tile([128, nb], F32)
            nc.sync.dma_start(out=lab_s[:], in_=labt[:])
            ncl_s = sing.tile([128, nb], F32)
            m0_s = sing.tile([128, nb], F32)
            off_s = sing.tile([128, nb], F32)
            n8_s = sing.tile([128, nb, 8], F32)
            zero8 = sing.tile([128, 8], F32)
            nc.vector.memset(zero8[:], 0.0)
            one8 = sing.tile([128, 8], F32)
            nc.vector.memset(one8[:], 1.0)
            iota8 = sing.tile([128, 8], F32)  # 1..8 per partition
            nc.vector.tensor_tensor_scan(
                iota8[:], one8[:], zero8[:], 0.0, AF.add, AF.add)

            for b in range(nb):
                V = work.tile([128, W], F32, tag="V")
                nc.sync.dma_start(out=V[:], in_=vt[b])
                # spk' = (v < 0) = Sign(Relu(-v))
                SPr = work.tile([128, W], F32, tag="SPr")
                nc.scalar.activation(SPr[:], V[:], ACT.Relu,
                                     bias=0.0, scale=-1.0)
                SP = work.tile([128, W], F32, tag="SP")
                nc.scalar.activation(SP[:], SPr[:], ACT.Sign,
                                     bias=0.0, scale=1.0)
                # tsls scan: state = spk'*(state+1), init 9 (col0 pad -> 10)
                TSL = work.tile([128, W], F32, tag="TSL")
                nc.vector.tensor_tensor_scan(
                    TSL[:], SP[:], SP[:], 9.0, AF.mult, AF.add)
                # is_start[j] = (tsls[j-1] - 10 >= tsls[j]); sum-accum = nclus
                IST = work.tile([128, 528], F32, tag="IST")
                nc.vector.scalar_tensor_tensor(
                    IST[:], TSL[:, 0:528], -10.0, TSL[:, 1:529],
                    AF.add, AF.is_ge, accum_out=ncl_s[:, b:b + 1])
                nc.gpsimd.memset(IST[:, 506:507], 1.0)
                # mask*1e30: spike within [t-10, t+10] <=> tsls[t+10] <= 20
                MK = work.tile([128, 500], F32, tag="MK")
                nc.vector.tensor_scalar(
                    MK[:], TSL[:, 11:511], 20.0, BIG, AF.is_le, AF.mult)
                # w0 = v - mask*1e30 ; m0 = max(w0)
                W0 = work.tile([128, 500], F32, tag="W0")
                nc.gpsimd.tensor_tensor(W0[:], V[:, 1:501], MK[:], AF.subtract)
                nc.vector.tensor_reduce(
                    m0_s[:, b:b + 1], W0[:], mybir.AxisListType.X, AF.max)
                # block level (48 blocks of 11 over t=0..527)
                BMX = work.tile([128, 48], F32, tag="BMX")
                nc.vector.tensor_reduce(
                    BMX[:], V[:, 1:529].rearrange("p (a b) -> p a b", b=11),
                    mybir.AxisListType.X, AF.max)
                BST = work.tile([128, 48], F32, tag="BST")
                nc.vector.tensor_reduce(
                    BST[:], IST.rearrange("p (a b) -> p a b", b=11),
                    mybir.AxisListType.X, AF.max)
                nc.vector.tensor_scalar(
                    off_s[:, b:b + 1], BST[:, 0:1], -1.0, 1.0,
                    AF.mult, AF.add)
                # block segmented running max: state = max(Rb*state, bmax)
                Rb = work.tile([128, 47], F32, tag="Rb")
                nc.scalar.activation(Rb[:], BST[:, 0:47], ACT.Copy,
                                     bias=1.0, scale=-1.0)
                BREC = work.tile([128, 47], F32, tag="BREC")
                nc.vector.tensor_tensor_scan(
                    BREC[:], Rb[:], BMX[:, 0:47], -BIG, AF.mult, AF.max)
                # z = (SENT*bend - SENT) - brec*bend  (= -rec at cluster ends)
                T1 = work.tile([128, 47], F32, tag="T1")
                nc.gpsimd.tensor_tensor(
                    T1[:], BREC[:], BST[:, 1:48], AF.mult)
                Q = work.tile([128, 47], F32, tag="Q")
                nc.scalar.activation(Q[:], BST[:, 1:48], ACT.Copy,
                                     bias=-SENT, scale=SENT)
                Z = work.tile([128, 47], F32, tag="Z")
                nc.gpsimd.tensor_tensor(Z[:], Q[:], T1[:], AF.subtract)
                # top8 descending = [fake(+)?, -rec_(1), ..., -rec_(k), -64..]
                M8 = work.tile([128, 8], F32, tag="M8")
                nc.vector.max(M8[:], Z[:])
                nc.scalar.dma_start(out=n8_s[:, b, :], in_=M8[:])

            # ---- end phase on [128, nb] stats (n8 holds -rec values) ----
            dif = sing.tile([128, nb], F32)
            nc.vector.tensor_tensor(dif[:], ncl_s[:], lab_s[:], AF.subtract)
            dD = sing.tile([128, nb], F32)
            nc.vector.tensor_scalar(dD[:], dif[:], 1.0, None, AF.max)
            mm = sing.tile([128, nb], F32)
            nc.vector.tensor_tensor(mm[:], dD[:], ncl_s[:], AF.min)
            mmo = sing.tile([128, nb], F32)
            nc.vector.tensor_tensor(mmo[:], mm[:], off_s[:], AF.add)
            nco = sing.tile([128, nb], F32)
            nc.vector.tensor_tensor(nco[:], ncl_s[:], off_s[:], AF.add)
            vmn_s = sing.tile([128, nb], F32)   # = -vmax
            sumB_s = sing.tile([128, nb], F32)  # = -(fake + mm smallest recs)
            sumC_s = sing.tile([128, nb], F32)  # = -fake
            scrA = sing.tile([128, nb, 8], F32)
            scrB = sing.tile([128, nb, 8], F32)
            scrC = sing.tile([128, nb, 8], F32)
            for b in range(nb):
                nc.vector.scalar_tensor_tensor(
                    scrA[:, b, :], iota8[:], nco[:, b:b + 1], n8_s[:, b, :],
                    AF.is_equal, AF.mult, accum_out=vmn_s[:, b:b + 1])
                nc.vector.scalar_tensor_tensor(
                    scrB[:, b, :], iota8[:], mmo[:, b:b + 1], n8_s[:, b, :],
                    AF.is_le, AF.mult, accum_out=sumB_s[:, b:b + 1])
                nc.vector.scalar_tensor_tensor(
                    scrC[:, b, :], iota8[:], off_s[:, b:b + 1], n8_s[:, b, :],
                    AF.is_le, AF.mult, accum_out=sumC_s[:, b:b + 1])
            ds = sing.tile([128, nb], F32)   # sum of mm smallest recs
            nc.vector.tensor_tensor(ds[:], sumC_s[:], sumB_s[:], AF.subtract)
            rcp = sing.tile([128, nb], F32)
            nc.vector.reciprocal(rcp[:], dD[:])
            dls = sing.tile([128, nb], F32)
            nc.vector.tensor_tensor(dls[:], ds[:], rcp[:], AF.mult)
            fz = sing.tile([128, nb], F32)
            nc.vector.tensor_scalar(fz[:], m0_s[:], -1e29, None, AF.is_lt)
            f1 = sing.tile([128, nb], F32)
            nc.vector.tensor_scalar(f1[:], fz[:], -1.0, 1.0, AF.mult, AF.add)
            ea = sing.tile([128, nb], F32)   # fz * (-vmax)
            nc.vector.tensor_tensor(ea[:], fz[:], vmn_s[:], AF.mult)
            eb = sing.tile([128, nb], F32)   # (1-fz) * m0
            nc.vector.tensor_tensor(eb[:], f1[:], m0_s[:], AF.mult)
            exn = sing.tile([128, nb], F32)  # = -(excess)
            nc.vector.tensor_tensor(exn[:], ea[:], eb[:], AF.add)
            cgt = sing.tile([128, nb], F32)
            nc.vector.tensor_tensor(cgt[:], lab_s[:], ncl_s[:], AF.is_gt)
            clt = sing.tile([128, nb], F32)
            nc.vector.tensor_tensor(clt[:], lab_s[:], ncl_s[:], AF.is_lt)
            c1 = sing.tile([128, nb], F32)
            nc.vector.tensor_tensor(c1[:], cgt[:], exn[:], AF.mult)
            c2 = sing.tile([128, nb], F32)
            nc.vector.tensor_tensor(c2[:], clt[:], dls[:], AF.mult)
            ctr = sing.tile([128, nb], F32)
            nc.vector.tensor_tensor(ctr[:], c2[:], c1[:], AF.subtract)
            csum = sing.tile([128, 1], F32)
            nc.vector.tensor_reduce(csum[:], ctr[:], mybir.AxisListType.X, AF.add)
            nc.sync.dma_start(out=out[:, 0:nb], in_=ncl_s[:])
            nc.sync.dma_start(out=out[:, nb:nb + 1], in_=csum[:])
    return nc


def _fix_wait_overflow(nc):
    """walrus embeds at most 1 sync-wait in standard TPB/DMA instruction
    structs and none in the custom DVE ops (InstMax/InstMaxIndex); move
    overflow waits onto injected same-engine no-fuse NoOps."""
    zero_wait = (mybir.InstMax, mybir.InstMaxIndex)
    fid = 0
    for f in nc.m.functions:
        for bb in f.blocks:
            new = []
            for ins in bb.instructions:
                si = getattr(ins, "sync_info", None)
                if (si is None or not si.on_wait
                        or isinstance(ins, mybir.InstNoOp)):
                    new.append(ins)
                    continue
                cap = 0 if isinstance(ins, zero_wait) else 1
                waits = list(si.on_wait)
                if len(waits) <= cap:
                    new.append(ins)
                    continue
                keep = waits[-cap:] if cap else []
                for w in (waits[:-cap] if cap else waits):
                    nop = mybir.InstNoOp(name=f"I-fixw-{fid}", ins=[], outs=[])
                    fid += 1
                    nop.engine = ins.engine
                    nop.bass_nofuse = True
                    nop.sync_info = mybir.SyncInfo(on_wait=[w], on_update=[])
                    new.append(nop)
                ins.sync_info = mybir.SyncInfo(
                    on_wait=keep, on_update=si.on_update)
                new.append(ins)
            bb.instructions[:] = new
    return nc


_NC = None


def _get_nc():
    global _NC
    if _NC is None:
        _NC = _fix_wait_overflow(_build(NB))
    return _NC


def _host_pre(vmem, labels):
    in_maps = []
    for c in range(NCORES):
        vs = vmem[c * NB:(c + 1) * NB]                        # [NB, 500, 128]
        vt = np.full((NB, 128, W), -1.0, np.float32)
        vt[:, :, 1:501] = vs.transpose(0, 2, 1)               # [NB, 128, 500]
        ls = labels[c * NB:(c + 1) * NB].astype(np.float32)   # [NB, 128]
        labt = np.ascontiguousarray(ls.T)                     # [128, NB]
        in_maps.append({"vt": vt, "labt": labt})
    return in_maps


def _host_post(results):
    spikes = []
    loss = np.float32(0.0)
    for c in range(NCORES):
        o = np.asarray(results[c]["out"])                     # [128, NB+1]
        spikes.append(np.ascontiguousarray(o[:, :NB].T))
        loss += o[:, NB].sum(dtype=np.float32)
    spike = np.concatenate(spikes, axis=0).astype(np.float32)  # [256, 128]
    return np.float32(loss), spike


def kernel(vmem, vlastmem, labels_t, ratio):
    vmem = np.asarray(vmem, dtype=np.float32)
    labels = np.asarray(labels_t)
    in_maps = _host_pre(vmem, labels)
    res = run_bass_kernel_spmd(_get_nc(), in_maps, list(range(NCORES)))
    return _host_post(res.results)


def profile_once(vmem, vlastmem, labels_t, ratio, tmpdir=None):
    """Run once with NTFF tracing; returns HW exec time in ns (or None)."""
    vmem = np.asarray(vmem, dtype=np.float32)
    in_maps = _host_pre(vmem, np.asarray(labels_t))
    res = run_bass_kernel_spmd(
        _get_nc(), in_maps, list(range(NCORES)), trace=True, tmpdir=tmpdir)
    return res.exec_time_ns


# revision 22
# speedup vs baseline: 1.2904x; 1.2904x over previous
"""Trainium2 Bass kernel for the ATCA/TCA spiking cluster loss.

Contract: kernel(**inputs) takes FULL inputs (vmem [256,500,128] f32,
vlastmem [256,500,128] f32 (unused by the math), labels_t [256,128] i32,
ratio scalar (unused)) and returns (loss: f32 scalar, spike_output
[256,128] f32), exactly like the reference.

Strategy (data-parallel over batch, 8 NeuronCores):
  - host: shard vmem/labels along batch (32 batches/core); transpose each
    batch slice to [128 neurons, 532 t] (time on the SBUF free axis; col 0
    and cols 501..531 are -1 pads = "no spike", covering t = -1 and
    t = 500..527 so all shifted views and 11-blocks stay in range)
  - device (per core, per batch tile [128, 532]):
      spk' = (v < 0) = Sign(Relu(-v))
      tsls = scan state = spk'*(state+1)        (time since last spike)
      is_start[t] = (tsls[t-1] - 10 >= tsls[t]); nclus = sum(is_start)
      mask[t] = (tsls[t+10] <= 20)              (spike within [t-10, t+10])
      m0 = max(v - 1e30*mask); full0 = m0 < -1e29
      excess = full0 ? vmax : -m0               (valid since every pair spikes:
                                                 the argmax win lies inside mask)
      cluster maxima at BLOCK level: gaps between clusters are >= 11 steps,
      so an 11-block intersects at most one cluster. bmax = per-block max of
      v (48 blocks over t=0..527); bstart = per-block OR of is_start, with a
      forced fake start at t=506 (block 46 boundary) closing the last real
      cluster; a 47-wide segmented-max scan over bmax + top8 of the
      end-masked values yields the per-cluster maxima (negated, ascending),
      with at most one positive "fake" entry from the pre-first-spike region
      offset-corrected during selection (off = 1 - bstart[0]).
      contrib = label > nclus ? excess : (label < nclus ? deficit : 0)
  - device reduces contribs to [128] partials; host sums 8x128 partials and
    concatenates spike counts.

Engine split: DVE gets the scans, compares and reductions; Pool (gpsimd)
gets the plain tensor_tensor ops; ACT gets the affine/Relu/Sign ops; spare
DMA queues move the tiny top8 tiles into the stats buffer. walrus on this
toolchain embeds at most one sync-wait per TPB/DMA instruction (none on
InstMax); _fix_wait_overflow moves overflow waits onto injected NoOps.
"""
import sys

sys.path.insert(0, "/opt/trn_rl_repo")

import numpy as np
import concourse.bass as bass
import concourse.tile as tile
from concourse import mybir
from concourse.bass_utils import run_bass_kernel_spmd

AF = mybir.AluOpType
F32 = mybir.dt.float32

B, T, N = 256, 500, 128
NCORES = 8
NB = B // NCORES  # 32 batch elements per core
SENT = 64.0
BIG = 1e30
W = 532           # padded time width: col0 = t=-1, cols 1..529+ = t 0..527+


def _build(nb: int) -> bass.Bass:
    ACT = mybir.ActivationFunctionType
    nc = bass.Bass()
    vt = nc.dram_tensor("vt", [nb, 128, W], F32, kind="ExternalInput")
    labt = nc.dram_tensor("labt", [128, nb], F32, kind="ExternalInput")
    out = nc.dram_tensor("out", [128, nb + 1], F32, kind="ExternalOutput")

    with tile.TileContext(nc) as tc:
        with (
            tc.tile_pool(name="work", bufs=3) as work,
            tc.tile_pool(name="sing", bufs=1) as sing,
        ):
            lab_s = sing.tile([128, nb], F32)
            nc.sync.dma_start(out=lab_s[:], in_=labt[:])
            ncl_s = sing.tile([128, nb], F32)
            m0_s = sing.tile([128, nb], F32)
            off_s = sing.tile([128, nb], F32)
            n8_s = sing.tile([128, nb, 8], F32)
            zero8 = sing.tile([128, 8], F32)
            nc.vector.memset(zero8[:], 0.0)
            one8 = sing.tile([128, 8], F32)
            nc.vector.memset(one8[:], 1.0)
            iota8 = sing.tile([128, 8], F32)  # 1..8 per partition
            nc.vector.tensor_tensor_scan(
                iota8[:], one8[:], zero8[:], 0.0, AF.add, AF.add)

            CPB = min(4, nb)  # batch cells per mega-tile
            for g in range(nb // CPB):
                b0 = g * CPB
                V = work.tile([128, CPB, W], F32, tag="V")
                nc.sync.dma_start(out=V[:], in_=vt[b0:b0 + CPB].rearrange("c p w -> p c w"))
                Vf = V.rearrange("p c w -> p (c w)")
                # spk' = (v < 0) = Sign(Relu(-v))
                SPr = work.tile([128, CPB * W], F32, tag="SPr")
                nc.scalar.activation(SPr[:], Vf[:], ACT.Relu,
                                     bias=0.0, scale=-1.0)
                SP = work.tile([128, CPB * W], F32, tag="SP")
                nc.scalar.activation(SP[:], SPr[:], ACT.Sign,
                                     bias=0.0, scale=1.0)
                # tsls scan across all cells (inter-cell pads >= 10 no-spike
                # cols make the state semantically fresh at each cell start)
                TSL = work.tile([128, CPB * W], F32, tag="TSL")
                nc.vector.tensor_tensor_scan(
                    TSL[:], SP[:], SP[:], 9.0, AF.mult, AF.add)
                TSL3 = TSL.rearrange("p (c w) -> p c w", c=CPB)
                # is_start per cell over t=0..527
                IST = work.tile([128, CPB, 528], F32, tag="IST")
                nc.vector.scalar_tensor_tensor(
                    IST[:], TSL3[:, :, 0:528], -10.0, TSL3[:, :, 1:529],
                    AF.add, AF.is_ge)
                nc.gpsimd.memset(IST[:, :, 506:507], 1.0)
                # mask*1e30 per cell
                MK = work.tile([128, CPB, 500], F32, tag="MK")
                nc.vector.tensor_scalar(
                    MK[:], TSL3[:, :, 11:511], 20.0, BIG, AF.is_le, AF.mult)
                W0 = work.tile([128, CPB, 500], F32, tag="W0")
                nc.gpsimd.tensor_tensor(
                    W0[:], V[:, :, 1:501], MK[:], AF.subtract)
                nc.vector.tensor_reduce(
                    m0_s[:, b0:b0 + CPB], W0[:], mybir.AxisListType.X, AF.max)
                # block level: 48 blocks of 11 per cell
                BMX = work.tile([128, CPB, 48], F32, tag="BMX")
                nc.vector.tensor_reduce(
                    BMX[:], V[:, :, 1:529].rearrange("p c (a b) -> p c a b",
                                                     b=11),
                    mybir.AxisListType.X, AF.max)
                BST = work.tile([128, CPB, 48], F32, tag="BST")
                nc.vector.tensor_reduce(
                    BST[:], IST.rearrange("p c (a b) -> p c a b", b=11),
                    mybir.AxisListType.X, AF.max)
                # nclus = sum(bstart) - 1 (the -1 applied in the end phase)
                nc.vector.tensor_reduce(
                    ncl_s[:, b0:b0 + CPB], BST[:], mybir.AxisListType.X,
                    AF.add)
                nc.vector.tensor_scalar(
                    off_s[:, b0:b0 + CPB], BST[:, :, 0], -1.0, 1.0,
                    AF.mult, AF.add)
                # merged block segmented max (state crosses cells only through
                # all-pad blocks; any contamination lands in the fake entry,
                # which stays positive and is offset-skipped)
                BSTf = BST.rearrange("p c a -> p (c a)")
                Rb = work.tile([128, CPB * 48], F32, tag="Rb")
                nc.scalar.activation(Rb[:], BSTf[:], ACT.Copy,
                                     bias=1.0, scale=-1.0)
                BMXf = BMX.rearrange("p c a -> p (c a)")
                BREC = work.tile([128, CPB * 48], F32, tag="BREC")
                nc.vector.tensor_tensor_scan(
                    BREC[:], Rb[:], BMXf[:], -BIG, AF.mult, AF.max)
                BREC3 = BREC.rearrange("p (c a) -> p c a", c=CPB)
                BST3 = BST
                T1 = work.tile([128, CPB, 47], F32, tag="T1")
                nc.gpsimd.tensor_tensor(
                    T1[:], BREC3[:, :, 0:47], BST3[:, :, 1:48], AF.mult)
                Q = work.tile([128, CPB, 47], F32, tag="Q")
                nc.scalar.activation(Q[:], BST3[:, :, 1:48], ACT.Copy,
                                     bias=-SENT, scale=SENT)
                Z = work.tile([128, CPB, 47], F32, tag="Z")
                nc.gpsimd.tensor_tensor(Z[:], Q[:], T1[:], AF.subtract)
                for c in range(CPB):
                    M8 = work.tile([128, 8], F32, tag=f"M8_{c}")
                    nc.vector.max(M8[:], Z[:, c, :])
                    nc.scalar.dma_start(out=n8_s[:, b0 + c, :], in_=M8[:])

            # ---- end phase on [128, nb] stats (n8 holds -rec values) ----
            # ncl_s currently = sum(bstart) = nclus + 1 (the forced fake)
            nc.vector.tensor_scalar(ncl_s[:], ncl_s[:], -1.0, None, AF.add)
            dif = sing.tile([128, nb], F32)
            nc.vector.tensor_tensor(dif[:], ncl_s[:], lab_s[:], AF.subtract)
            dD = sing.tile([128, nb], F32)
            nc.vector.tensor_scalar(dD[:], dif[:], 1.0, None, AF.max)
            mm = sing.tile([128, nb], F32)
            nc.vector.tensor_tensor(mm[:], dD[:], ncl_s[:], AF.min)
            mmo = sing.tile([128, nb], F32)
            nc.vector.tensor_tensor(mmo[:], mm[:], off_s[:], AF.add)
            nco = sing.tile([128, nb], F32)
            nc.vector.tensor_tensor(nco[:], ncl_s[:], off_s[:], AF.add)
            vmn_s = sing.tile([128, nb], F32)   # = -vmax
            sumB_s = sing.tile([128, nb], F32)  # = -(fake + mm smallest recs)
            sumC_s = sing.tile([128, nb], F32)  # = -fake
            scrA = sing.tile([128, nb, 8], F32)
            scrB = sing.tile([128, nb, 8], F32)
            scrC = sing.tile([128, nb, 8], F32)
            for b in range(nb):
                nc.vector.scalar_tensor_tensor(
                    scrA[:, b, :], iota8[:], nco[:, b:b + 1], n8_s[:, b, :],
                    AF.is_equal, AF.mult, accum_out=vmn_s[:, b:b + 1])
                nc.vector.scalar_tensor_tensor(
                    scrB[:, b, :], iota8[:], mmo[:, b:b + 1], n8_s[:, b, :],
                    AF.is_le, AF.mult, accum_out=sumB_s[:, b:b + 1])
                nc.vector.scalar_tensor_tensor(
                    scrC[:, b, :], iota8[:], off_s[:, b:b + 1], n8_s[:, b, :],
                    AF.is_le, AF.mult, accum_out=sumC_s[:, b:b + 1])
            ds = sing.tile([128, nb], F32)   # sum of mm smallest recs
            nc.vector.tensor_tensor(ds[:], sumC_s[:], sumB_s[:], AF.subtract)
            rcp = sing.tile([128, nb], F32)
            nc.vector.reciprocal(rcp[:], dD[:])
            dls = sing.tile([128, nb], F32)
            nc.vector.tensor_tensor(dls[:], ds[:], rcp[:], AF.mult)
            fz = sing.tile([128, nb], F32)
            nc.vector.tensor_scalar(fz[:], m0_s[:], -1e29, None, AF.is_lt)
            f1 = sing.tile([128, nb], F32)
            nc.vector.tensor_scalar(f1[:], fz[:], -1.0, 1.0, AF.mult, AF.add)
            ea = sing.tile([128, nb], F32)   # fz * (-vmax)
            nc.vector.tensor_tensor(ea[:], fz[:], vmn_s[:], AF.mult)
            eb = sing.tile([128, nb], F32)   # (1-fz) * m0
            nc.vector.tensor_tensor(eb[:], f1[:], m0_s[:], AF.mult)
            exn = sing.tile([128, nb], F32)  # = -(excess)
            nc.vector.tensor_tensor(exn[:], ea[:], eb[:], AF.add)
            cgt = sing.tile([128, nb], F32)
            nc.vector.tensor_tensor(cgt[:], lab_s[:], ncl_s[:], AF.is_gt)
            clt = sing.tile([128, nb], F32)
            nc.vector.tensor_tensor(clt[:], lab_s[:], ncl_s[:], AF.is_lt)
            c1 = sing.tile([128, nb], F32)
            nc.vector.tensor_tensor(c1[:], cgt[:], exn[:], AF.mult)
            c2 = sing.tile([128, nb], F32)
            nc.vector.tensor_tensor(c2[:], clt[:], dls[:], AF.mult)
            ctr = sing.tile([128, nb], F32)
            nc.vector.tensor_tensor(ctr[:], c2[:], c1[:], AF.subtract)
            csum = sing.tile([128, 1], F32)
            nc.vector.tensor_reduce(csum[:], ctr[:], mybir.AxisListType.X, AF.add)
            nc.sync.dma_start(out=out[:, 0:nb], in_=ncl_s[:])
            nc.sync.dma_start(out=out[:, nb:nb + 1], in_=csum[:])
    return nc


def _fix_wait_overflow(nc):
    """walrus embeds at most 1 sync-wait in standard TPB/DMA instruction
    structs and none in the custom DVE ops (InstMax/InstMaxIndex); move
    overflow waits onto injected same-engine no-fuse NoOps."""
    zero_wait = (mybir.InstMax, mybir.InstMaxIndex)
    fid = 0
    for f in nc.m.functions:
        for bb in f.blocks:
            new = []
            for ins in bb.instructions:
                si = getattr(ins, "sync_info", None)
                if (si is None or not si.on_wait
                        or isinstance(ins, mybir.InstNoOp)):
                    new.append(ins)
                    continue
                cap = 0 if isinstance(ins, zero_wait) else 1
                waits = list(si.on_wait)
                if len(waits) <= cap:
                    new.append(ins)
                    continue
                keep = waits[-cap:] if cap else []
                for w in (waits[:-cap] if cap else waits):
                    nop = mybir.InstNoOp(name=f"I-fixw-{fid}", ins=[], outs=[])
                    fid += 1
                    nop.engine = ins.engine
                    nop.bass_nofuse = True
                    nop.sync_info = mybir.SyncInfo(on_wait=[w], on_update=[])
                    new.append(nop)
                ins.sync_info = mybir.SyncInfo(
                    on_wait=keep, on_update=si.on_update)
                new.append(ins)
            bb.instructions[:] = new
    return nc


_NC = None


def _get_nc():
    global _NC
    if _NC is None:
        _NC = _fix_wait_overflow(_build(NB))
    return _NC


def _host_pre(vmem, labels):
    in_maps = []
    for c in range(NCORES):
        vs = vmem[c * NB:(c + 1) * NB]                        # [NB, 500, 128]
        vt = np.full((NB, 128, W), -1.0, np.float32)
        vt[:, :, 1:501] = vs.transpose(0, 2, 1)               # [NB, 128, 500]
        ls = labels[c * NB:(c + 1) * NB].astype(np.float32)   # [NB, 128]
        labt = np.ascontiguousarray(ls.T)                     # [128, NB]
        in_maps.append({"vt": vt, "labt": labt})
    return in_maps


def _host_post(results):
    spikes = []
    loss = np.float32(0.0)
    for c in range(NCORES):
        o = np.asarray(results[c]["out"])                     # [128, NB+1]
        spikes.append(np.ascontiguousarray(o[:, :NB].T))
        loss += o[:, NB].sum(dtype=np.float32)
    spike = np.concatenate(spikes, axis=0).astype(np.float32)  # [256, 128]
    return np.float32(loss), spike


def kernel(vmem, vlastmem, labels_t, ratio):
    vmem = np.asarray(vmem, dtype=np.float32)
    labels = np.asarray(labels_t)
    in_maps = _host_pre(vmem, labels)
    res = run_bass_kernel_spmd(_get_nc(), in_maps, list(range(NCORES)))
    return _host_post(res.results)


def profile_once(vmem, vlastmem, labels_t, ratio, tmpdir=None):
    """Run once with NTFF tracing; returns HW exec time in ns (or None)."""
    vmem = np.asarray(vmem, dtype=np.float32)
    in_maps = _host_pre(vmem, np.asarray(labels_t))
    res = run_bass_kernel_spmd(
        _get_nc(), in_maps, list(range(NCORES)), trace=True, tmpdir=tmpdir)
    return res.exec_time_ns
